# revision 33
# baseline (speedup 1.0000x reference)
"""Trainium2 Bass kernel for nn_Block2x2DiagProduct (butterfly product).

Strategy:
  Stages 1..9 of the butterfly (all with block size <= 512) compose into
  blockdiag(R, R) with a single dense 512x512 matrix R shared by both
  halves (parameters are shared across blocks within each factor). The
  final stage (block size 1024) is a columnwise 2x2 butterfly:

      out[:, k]     = A[k]*y[:, k] + B[k]*y[:, 512+k]
      out[:, 512+k] = C[k]*y[:, k] + D[k]*y[:, 512+k]

  where y = x @ blockdiag(R^T, R^T). So the device kernel is two K=512
  float32r matmuls per row tile (PE) plus six columnwise multiply/adds
  (split across Vector and GpSimd, with Scalar doing the PSUM->SBUF
  staging). This halves the PE matmul work vs composing one dense
  1024x1024 matrix, moving the peeled stage to otherwise-idle engines.

  R is composed on the host in float64 (9 einsums over a 512x512
  identity). Sharding: pure data parallel — batch dim of x split across
  8 cores; R^T (1 MiB) and the stage-0 coefficients are replicated.

  Per-core per 128-row tile of x:
    - HWDGE DMA in; PE-transposes the 8 [128,128] feature chunks 4-up
      into [128,512] PSUM tiles (matmul contracts along partitions, so
      x needs features on partitions); Scalar-engine casts move them to
      SBUF as float32r (full-rate on PE, vs 1/4-rate plain fp32).
    - 8 accumulating float32r matmuls -> y_lo, y_hi in PSUM.
    - Butterfly: Vector computes A*y_lo + B*y_hi (reading PSUM), Scalar
      stages y_lo/y_hi to SBUF, GpSimd computes C*y_lo + D*y_hi (GpSimd
      cannot read PSUM), both into the output tile; HWDGE DMA out.
"""

import os
import sys

for _p in ("/opt/trn_rl_repo", "/root/.axon_site/_ro/trn_rl_repo"):
    if os.path.isdir(_p) and _p not in sys.path:
        sys.path.insert(0, _p)

import numpy as np

import concourse.bacc as bacc
import concourse.bass as bass
import concourse.mybir as mybir
from concourse.bass_utils import run_bass_kernel_spmd
from concourse.masks import make_identity
from concourse.tile import TileContext

SIZE = 1024
HALF = SIZE // 2
M = 10  # number of butterfly factors
N_CORES = 8
P = 128
KC = HALF // P  # 4 contraction chunks per half

# Results of the last device run (for the test harness).
last_exec_time_ns = None
last_mean_exec_time_ns = None

_nc_cache = {}


def _compose_w1t(params):
    """Compose butterfly stages 1..9 into W1t (512x512, f64) such that
    y_half = x_half @ W1t for each 512 half. Both halves share W1t because
    each factor's parameters are shared across its blocks."""
    w = np.eye(HALF, dtype=np.float64)
    for i in reversed(range(1, M)):
        s = SIZE >> i
        y = w.reshape(HALF, HALF // s, 2, s // 2)
        w = np.einsum(
            "ijk,bnjk->bnik", params[i].astype(np.float64), y
        ).reshape(HALF, HALF)
    return w


def _build_nc(rows):
    f32 = mybir.dt.float32
    f32r = mybir.dt.float32r
    nb = rows // P

    # Bacc (not raw Bass): its finalize() pipeline splits multi-sem waits
    # into EventSemaphore instructions (HW allows 1 sync-wait per inst).
    nc = bacc.Bacc(None, target_bir_lowering=False)
    x_d = nc.dram_tensor("x", [rows, SIZE], f32, kind="ExternalInput")
    w_d = nc.dram_tensor("w", [HALF, HALF], f32, kind="ExternalInput")
    coef_d = nc.dram_tensor("coef", [P, 4, HALF], f32, kind="ExternalInput")
    o_d = nc.dram_tensor("o", [rows, SIZE], f32, kind="ExternalOutput")

    with TileContext(nc) as tc:
        with (
            tc.tile_pool(name="const", bufs=1) as const_pool,
            tc.tile_pool(name="xin", bufs=8) as xpool,
            tc.tile_pool(name="xt", bufs=4) as xtpool,
            tc.tile_pool(name="stage", bufs=6) as spool,
            tc.tile_pool(name="osb", bufs=4) as opool,
            tc.tile_pool(name="tpsum", bufs=4, space="PSUM") as tpsum,
            tc.tile_pool(name="mpsum", bufs=4, space="PSUM") as mpsum,
        ):
            ident = const_pool.tile([P, P], f32)
            make_identity(nc, ident[:])
            # Dummy PE op consuming the identity: walrus allows only one
            # sync-wait on (transpose-)matmuls, and without this the first
            # real transpose would need two (identity-ready + x-DMA).
            pst0 = tpsum.tile([P, P], f32, name="pst_warm", tag="pst")
            nc.tensor.transpose(pst0[:], ident[:], ident[:])

            # W1t resident in SBUF: partition p, chunk c holds W1t[c*128+p, :].
            # SWDGE + per-chunk loads: doesn't serialize the HWDGE x loads,
            # and chunk 0's float32r cast is ready early.
            w_sb = const_pool.tile([P, KC, HALF], f32)
            w_sbr = const_pool.tile([P, KC, HALF], f32r)
            for c in range(KC):
                # ACT HWDGE queue: runs in parallel with the x loads on the
                # SP queue (the store stream it shares is idle at startup).
                nc.scalar.dma_start(
                    out=w_sb[:, c, :], in_=w_d[c * P : (c + 1) * P, :]
                )
                # FP32r matmul operands must be produced rounded-to-FP32r.
                nc.vector.tensor_copy(out=w_sbr[:, c, :], in_=w_sb[:, c, :])
            # Stage-0 coefficients A,B,C,D, pre-replicated across partitions.
            coef_sb = const_pool.tile([P, 4, HALF], f32)
            nc.scalar.dma_start(out=coef_sb[:], in_=coef_d[:, :, :])

            for bp in range(nb // 2):
                # Two 128-row tiles per DMA: 1 MiB transfers are the DMA
                # bandwidth sweet spot and halve the DMA op count. bufs=4
                # keeps the slot-WAW predecessor on the own HWDGE lane so
                # the load fits the DMA struct's sync-wait limit.
                x_sb = xpool.tile([P, 2, SIZE], f32)
                nc.sync.dma_start(
                    out=x_sb[:],
                    in_=x_d[bp * 2 * P : (bp + 1) * 2 * P, :].rearrange(
                        "(j p) f -> p j f", p=P
                    ),
                )
                o_sb = opool.tile([P, 2, SIZE], f32)
                for j in range(2):
                    # Transpose 8 chunks of [128b, 128f] -> [128f, 128b],
                    # 4 chunks per PSUM bank, one Scalar-engine cast each.
                    xts = []
                    for h in range(2):
                        pst = tpsum.tile(
                            [P, HALF], f32, tag="pst", name=f"pst{h}"
                        )
                        for c in range(KC):
                            k = KC * h + c
                            nc.tensor.transpose(
                                pst[:, c * P : (c + 1) * P],
                                x_sb[:, j, k * P : (k + 1) * P],
                                ident[:],
                            )
                        xt_h = xtpool.tile(
                            [P, HALF], f32r, tag="xt", name=f"xt{h}"
                        )
                        nc.scalar.copy(out=xt_h[:], in_=pst[:])
                        xts.append(xt_h)
                    # y_half[b, :] = sum_k x_half[b, k] * W1t[k, :]
                    psos = [
                        mpsum.tile([P, HALF], f32, tag="mm_psum", name=f"pso{h}")
                        for h in range(2)
                    ]
                    for c in range(KC):
                        for h in range(2):
                            nc.tensor.matmul(
                                psos[h][:],
                                xts[h][:, c * P : (c + 1) * P],
                                w_sbr[:, c, :],
                                start=(c == 0),
                                stop=(c == KC - 1),
                            )
                    # Peeled stage 0: out_lo = A*y_lo + B*y_hi, out_hi =
                    # C*y_lo + D*y_hi. Vector does all four multiplies
                    # straight from PSUM (GpSimd cannot read PSUM); GpSimd
                    # does the two adds from SBUF.
                    t0 = spool.tile([P, HALF], f32, tag="t0", name="t0")
                    t1 = spool.tile([P, HALF], f32, tag="t1", name="t1")
                    t2 = spool.tile([P, HALF], f32, tag="t2", name="t2")
                    t3 = spool.tile([P, HALF], f32, tag="t3", name="t3")
                    nc.vector.tensor_mul(t0[:], psos[0][:], coef_sb[:, 0, :])
                    nc.vector.tensor_mul(t1[:], psos[1][:], coef_sb[:, 1, :])
                    nc.vector.tensor_mul(t2[:], psos[0][:], coef_sb[:, 2, :])
                    nc.vector.tensor_mul(t3[:], psos[1][:], coef_sb[:, 3, :])
                    nc.gpsimd.tensor_add(o_sb[:, j, :HALF], t0[:], t1[:])
                    nc.gpsimd.tensor_add(o_sb[:, j, HALF:], t2[:], t3[:])
                # Store on the ACT HWDGE queue so loads (SP queue) and
                # stores stream through separate DMA queues.
                nc.scalar.dma_start(
                    out=o_d[bp * 2 * P : (bp + 1) * 2 * P, :].rearrange(
                        "(j p) f -> p j f", p=P
                    ),
                    in_=o_sb[:],
                )
    nc.finalize()
    return nc


def kernel(**inputs):
    global last_exec_time_ns, last_mean_exec_time_ns

    x = np.ascontiguousarray(np.asarray(inputs["x"], dtype=np.float32))
    params = [np.asarray(inputs[f"ABCD{i}"]) for i in range(M)]
    w1t = np.ascontiguousarray(_compose_w1t(params).astype(np.float32))
    abcd0 = params[0].astype(np.float32)  # (2, 2, 512)
    coef = np.ascontiguousarray(
        np.broadcast_to(
            abcd0.reshape(1, 4, HALF), (P, 4, HALF)
        ).astype(np.float32)
    )

    batch = x.shape[0]
    if batch % (N_CORES * 2 * P) != 0:
        # Shape outside the tiled layout this kernel hardcodes — fall back
        # to a host matmul (correct, just not accelerated).
        full = _compose_w1t(params)
        y_lo = x[:, :HALF].astype(np.float64) @ full
        y_hi = x[:, HALF:].astype(np.float64) @ full
        a, b = params[0][0, 0].astype(np.float64), params[0][0, 1].astype(
            np.float64
        )
        c, dd = params[0][1, 0].astype(np.float64), params[0][1, 1].astype(
            np.float64
        )
        return np.concatenate(
            [a * y_lo + b * y_hi, c * y_lo + dd * y_hi], axis=1
        ).astype(np.float32)
    rows = batch // N_CORES

    if rows not in _nc_cache:
        _nc_cache[rows] = _build_nc(rows)
    nc = _nc_cache[rows]

    in_maps = [
        {"x": x[i * rows : (i + 1) * rows], "w": w1t, "coef": coef}
        for i in range(N_CORES)
    ]
    res = run_bass_kernel_spmd(nc, in_maps, core_ids=list(range(N_CORES)))
    last_exec_time_ns = res.exec_time_ns
    last_mean_exec_time_ns = res.mean_exec_time_ns

    return np.concatenate([r["o"] for r in res.results], axis=0)
